# revision 15
# baseline (speedup 1.0000x reference)
"""GQA self-attention block (q/k/v proj + causal softmax attention + o proj)
on 8 trn2 NeuronCores.

Sharding: batch (2) x query-head-groups (4) -> 8 cores. Core c handles
batch b=c//4 and heads [8g, 8g+8) where g=c%4 (kv heads [2g, 2g+2)).
Each core computes a partial output [T, D] = ctx_heads @ o_proj_cols.T;
the host sums the 4 partials per batch (all-reduce done host-side).

v4: all matmul operands bf16 (fp32 PSUM accumulation); every stationary
is 128 columns so fast-weight-load keeps weight loads overlapped with
in-flight matmuls. V is projected directly into key-partition layout
(x-chunk stationary, V weights moving) - no transpose pass. All DRAM
tensors are host-packed to the exact SBUF tile layout so each DMA is
long contiguous runs (16-32KB descriptors): descriptor generation, not
bandwidth, gated the strided versions. Output is bf16, one 512KB DMA
per 128-token row chunk, upcast host-side.

4-round software pipeline over 512-token blocks; per round a:
attention for query block a (S -> exp -> PV chunk pipeline, exact
128-granular causal trim) interleaved with qkv projection of block a+1
and output projection of block a-1. ACT runs the exp stream (~145us,
the only engine with exp) plus early-round out-proj drains; DVE drains
PSUM, adds causal masks, reciprocals; gpsimd does normalize muls +
small DMAs. Inputs load on two DMA rings (SP: x + out, ACT: weights).
"""

import os
import sys
from collections import deque

sys.path.insert(0, "/opt/trn_rl_repo")

import numpy as np

import concourse.bass as bass
import concourse.tile as tile
from concourse import bacc, mybir
from concourse.bass_utils import run_bass_kernel_spmd

F32 = mybir.dt.float32
BF16 = mybir.dt.bfloat16
EXP = mybir.ActivationFunctionType.Exp

B, T, D = 2, 2048, 2048
HQ, HK = 32, 8
DH = D // HQ              # 64 head dim
N_CORES = 8
GROUPS = 4                # head groups per batch
QCOLS = D // GROUPS       # 512 q cols per core
KCOLS = (D // 4) // GROUPS  # 128 k cols per core (2 kv heads)
WCOLS = QCOLS + 2 * KCOLS   # 768
BLK = 512                 # token/query block
NBLK = T // BLK           # 4
KT = D // 128             # 16 contraction tiles
NEG = -480.0              # additive mask pre-scale (-60 after 1/8 scale)

_cache = {}


def _build():
    nc = bacc.Bacc("TRN2", target_bir_lowering=False, debug=False)

    # host-packed to SBUF tile layouts (partition-major, contiguous)
    xw_d = nc.declare_dram_parameter("xw", [128, NBLK, KT, BLK], BF16, isOutput=False)
    wq_d = nc.declare_dram_parameter("wq", [128, 6, KT, 128], BF16, isOutput=False)
    op_d = nc.declare_dram_parameter("op", [128, 4, D], BF16, isOutput=False)
    masks_d = nc.declare_dram_parameter("masks", [128, 2, 128], BF16, isOutput=False)
    out_d = nc.declare_dram_parameter("out", [16, 128, D], BF16, isOutput=True)
    rcscr_d = nc.dram_tensor("rcscratch", [16, 1024], F32)

    with tile.TileContext(nc) as tc:
        with (
            tc.tile_pool(name="pers", bufs=1) as pers,
            tc.tile_pool(name="work", bufs=2) as work,
            tc.tile_pool(name="psum", bufs=1, space="PSUM") as psum,
        ):
            # ---- persistent SBUF ----
            xsb = pers.tile([128, NBLK, KT, BLK], BF16, tag="xsb")
            wq_sb = pers.tile([128, 6, KT, 128], BF16, tag="wq")
            op_sb = pers.tile([128, 4, D], BF16, tag="op")
            masks_sb = pers.tile([128, 2, 128], BF16, tag="masks")
            qt = [pers.tile([128, T], BF16, tag=f"qt{m}", name=f"qt{m}")
                  for m in range(4)]
            kp = [pers.tile([128, T], BF16, tag=f"kp{k}", name=f"kp{k}")
                  for k in range(2)]
            # vs[kv][:, c, 0:64] = V rows for key chunk c, col 64 = ones
            # (softmax denominator), cols 65:128 zero so the PV stationary
            # is 128 wide (fast weight load).
            vs = [pers.tile([128, 16, 128], BF16, tag=f"vs{k}", name=f"vs{k}")
                  for k in range(2)]
            ctx = [pers.tile([128, T], BF16, tag=f"ctx{m}", name=f"ctx{m}")
                   for m in range(4)]

            # ---- input DMA waves: SP ring carries x, ACT ring weights ----
            nc.sync.dma_start(masks_sb, masks_d[:])
            for b in range(NBLK):
                nc.sync.dma_start(xsb[:, b], xw_d[:, b])
            nc.scalar.dma_start(wq_sb[:, 0:2], wq_d[:, 0:2])
            nc.scalar.dma_start(wq_sb[:, 2:6], wq_d[:, 2:6])
            nc.scalar.dma_start(op_sb, op_d[:])
            for kv in range(2):
                nc.gpsimd.memset(vs[kv], 0.0)
                nc.gpsimd.memset(vs[kv][:, :, 64:65], 1.0)

            # ---- phase-1 block builder (qkv projection of token block b) ----
            def p1_quanta(b):
                """Yield closures; each emits one PE work quantum."""
                bsl = slice(BLK * b, BLK * b + BLK)

                def group(m, bsl=bsl, b=b):
                    ps = psum.tile([128, BLK], F32, tag="small", bufs=2,
                                   name="p1ps")
                    for k in range(KT):
                        nc.tensor.matmul(
                            ps,
                            wq_sb[:, m, k, :],
                            xsb[:, b, k, :],
                            start=(k == 0),
                            stop=(k == KT - 1),
                        )
                    if m < 4:
                        nc.vector.tensor_copy(qt[m][:, bsl], ps)
                    else:
                        nc.vector.tensor_copy(kp[0][0:64, bsl], ps[0:64, :])
                        nc.vector.tensor_copy(kp[1][64:128, bsl], ps[64:128, :])
                        # mirror each kv head to the other partition half
                        nc.gpsimd.dma_start(kp[0][64:128, bsl], kp[0][0:64, bsl])
                        nc.gpsimd.dma_start(kp[1][0:64, bsl], kp[1][64:128, bsl])

                for m in range(5):
                    yield lambda m=m: group(m)

                # V directly in [key, vdim] layout: x chunk stationary,
                # V weight columns moving.
                def vchunk(c, b=b):
                    cq = c % 4
                    ps = psum.tile([128, 128], F32, tag="small", bufs=2,
                                   name="vps")
                    for k in range(KT):
                        nc.tensor.matmul(
                            ps,
                            xsb[:, b, k, 128 * cq : 128 * cq + 128],
                            wq_sb[:, 5, k, :],
                            start=(k == 0),
                            stop=(k == KT - 1),
                        )
                    nc.vector.tensor_copy(vs[0][:, c, 0:64], ps[:, 0:64])
                    nc.vector.tensor_copy(vs[1][:, c, 0:64], ps[:, 64:128])

                def vpair(c):
                    vchunk(c)
                    vchunk(c + 1)

                for c in range(4 * b, 4 * b + 4, 2):
                    yield lambda c=c: vpair(c)

            # ---- phase-3 block builder (output projection of block b) ----
            def p3_quanta(b, stage_eng):
                for t in range(4 * b, 4 * b + 4):
                    ostage = work.tile([128, 4, 512], BF16, tag="ostage",
                                       bufs=2, name="ostage")

                    def tile_out(t, rp, ostage=ostage):
                        tsl = slice(128 * t, 128 * t + 128)
                        ps = psum.tile([128, 512], F32, tag="small", bufs=2,
                                       name="p3ps")
                        for m in range(4):
                            nc.tensor.matmul(
                                ps,
                                ctx[m][:, tsl],
                                op_sb[:, m, 512 * rp : 512 * rp + 512],
                                start=(m == 0),
                                stop=(m == 3),
                            )
                        if stage_eng == "act":
                            nc.scalar.copy(ostage[:, rp, :], ps)
                        else:
                            nc.vector.tensor_copy(ostage[:, rp, :], ps)
                        if rp == 3:
                            nc.sync.dma_start(out_d[t], ostage)

                    for rp in range(4):
                        yield lambda t=t, rp=rp, ostage=ostage: tile_out(t, rp)

            # ---- prologue: qkv projection of block 0, inline ----
            for q in p1_quanta(0):
                q()

            # ---- rounds ----
            pending_norm = []

            def _normalize(cu, m, a, mul_eng):
                qsl = slice(BLK * a, BLK * a + BLK)
                den128 = work.tile([128, 8], F32, tag="d128", bufs=2,
                                   name="den128")
                nc.gpsimd.dma_start(den128, cu[64:65, :])
                rcp = work.tile([128, 8], F32, tag="rcp", bufs=2, name="rcp")
                nc.vector.reciprocal(rcp, den128)
                ma = m * 4 + a
                nc.gpsimd.dma_start(rcscr_d[ma : ma + 1, :], rcp)
                bcs = work.tile([64, 1024], F32, tag="bcs", bufs=2, name="bcs")
                nc.gpsimd.dma_start(
                    bcs, rcscr_d[ma : ma + 1, :].partition_broadcast(64)
                )
                tmpB = work.tile([64, 512], BF16, tag="tb", bufs=2, name="tmpB")
                if mul_eng == "dve":
                    nc.vector.tensor_mul(
                        ctx[m][0:64, qsl], cu[0:64, 0:512], bcs[:, 0:512]
                    )
                    nc.vector.tensor_mul(
                        tmpB, cu[0:64, 512:1024], bcs[:, 512:1024]
                    )
                else:
                    nc.gpsimd.tensor_mul(
                        ctx[m][0:64, qsl], cu[0:64, 0:512], bcs[:, 0:512]
                    )
                    nc.gpsimd.tensor_mul(
                        tmpB, cu[0:64, 512:1024], bcs[:, 512:1024]
                    )
                nc.gpsimd.dma_start(ctx[m][64:128, qsl], tmpB)

            for a in range(NBLK):
                last_round = a == NBLK - 1
                p1q = deque(p1_quanta(a + 1)) if a + 1 < NBLK else deque()
                p3q = (deque(p3_quanta(a - 1, "act" if a <= 2 else "dve"))
                       if a >= 1 else deque())
                nj = 4 * (a + 1)
                chunk_i = 0
                for m in range(4):
                    kv = m // 2
                    ctxAB = psum.tile([128, 1024], F32, tag="ctx", bufs=1,
                                      name="ctxAB")
                    pend = []

                    def flush_pv(n, m=m, kv=kv, ctxAB=ctxAB, pend=pend, nj=nj):
                        while len(pend) > n:
                            pE, pjc, plo = pend.pop(0)
                            for h2 in range(2):
                                nc.tensor.matmul(
                                    ctxAB[:, 512 * h2 + plo : 512 * h2 + 512],
                                    vs[kv][:, pjc, :],
                                    pE[:, h2, plo:512],
                                    start=(pjc == 0),
                                    stop=(pjc == nj - 1 and h2 == 1),
                                )

                    for jc in range(nj):
                        o = jc - 4 * a
                        lo = 128 * o if o >= 0 else 0
                        jsl = slice(128 * jc, 128 * jc + 128)
                        S = psum.tile([128, 2, 512], F32, tag="S", bufs=2,
                                      name="S")
                        for h2 in range(2):
                            nc.tensor.matmul(
                                S[:, h2, lo:512],
                                kp[kv][64 * h2 : 64 * h2 + 64, jsl],
                                qt[m][64 * h2 : 64 * h2 + 64,
                                      BLK * a + lo : BLK * a + 512],
                                start=True,
                                stop=True,
                                tile_position=(64 * h2, 0),
                            )
                        if o >= 0:
                            nc.vector.tensor_add(
                                S[:, :, lo : lo + 128],
                                S[:, :, lo : lo + 128],
                                masks_sb,
                            )
                        E = work.tile([128, 2, 512], BF16, tag="E", bufs=4,
                                      name="E")
                        nc.scalar.activation(
                            E[:, :, lo:512], S[:, :, lo:512], EXP, scale=0.125
                        )
                        pend.append((E, jc, lo))
                        flush_pv(2)
                        # interleave filler PE work (next-block qkv proj,
                        # prev-block out proj)
                        if p3q:
                            p3q.popleft()()
                        if p1q and chunk_i % 2 == 1:
                            p1q.popleft()()
                        chunk_i += 1

                    flush_pv(0)

                    # fast PSUM release; normalize runs deferred except in
                    # the last round (the tail's out-proj waits on it)
                    cu = work.tile([65, 1024], F32, tag="cu", bufs=3, name="cu")
                    nc.vector.tensor_copy(cu, ctxAB[0:65, :])
                    if last_round:
                        _normalize(cu, m, a, "dve")
                    else:
                        pending_norm.append(
                            lambda cu=cu, m=m, a=a: _normalize(cu, m, a, "gps")
                        )
                        while len(pending_norm) > 2:
                            pending_norm.pop(0)()

                # round end: flush fillers that later rounds depend on
                while p1q:
                    p1q.popleft()()
                while p3q:
                    p3q.popleft()()
                while pending_norm:
                    pending_norm.pop(0)()

            # ---- tail: output projection of block 3 ----
            for q in p3_quanta(NBLK - 1, "dve"):
                q()

    nc.compile()
    return nc


def _host_inputs(x, q_proj, k_proj, v_proj, o_proj):
    """Per-core input dicts (numpy, bf16, packed to SBUF tile layouts)."""
    import ml_dtypes

    bf16 = ml_dtypes.bfloat16
    masks = np.zeros((128, 2, 128), dtype=np.float32)
    jj = np.arange(128)[:, None]
    ii = np.arange(128)[None, :]
    tri = np.where(jj <= ii, 0.0, NEG)
    masks[:, 0, :] = tri
    masks[:, 1, :] = tri
    masks = masks.astype(bf16)

    # x[b].T is [D, T]; pack to [128p, NBLK, KT, BLK]
    xw = []
    for b in range(B):
        xT = np.ascontiguousarray(x[b].T).astype(bf16)
        xw.append(
            np.ascontiguousarray(
                xT.reshape(KT, 128, NBLK, BLK).transpose(1, 2, 0, 3)
            )
        )
    in_maps = []
    for c in range(N_CORES):
        b, g = divmod(c, GROUPS)
        wqkv = np.concatenate(
            [
                q_proj[QCOLS * g : QCOLS * g + QCOLS].T,
                k_proj[KCOLS * g : KCOLS * g + KCOLS].T,
                v_proj[KCOLS * g : KCOLS * g + KCOLS].T,
            ],
            axis=1,
        ).astype(bf16)  # [D, WCOLS]
        # pack to [128p, 6m, KT, 128]
        wq = np.ascontiguousarray(
            wqkv.reshape(KT, 128, 6, 128).transpose(1, 2, 0, 3)
        )
        opj = np.ascontiguousarray(
            o_proj[:, QCOLS * g : QCOLS * g + QCOLS].T
        ).astype(bf16)  # [QCOLS, D]
        op = np.ascontiguousarray(opj.reshape(4, 128, D).transpose(1, 0, 2))
        in_maps.append(
            {"xw": xw[b], "wq": wq, "op": op, "masks": masks}
        )
    return in_maps


def run(x, q_proj, k_proj, v_proj, o_proj, trace=False):
    """Run on hardware; returns (output [B,T,D] f32, BassKernelResults)."""
    if "nc" not in _cache:
        _cache["nc"] = _build()
    nc = _cache["nc"]
    in_maps = _host_inputs(x, q_proj, k_proj, v_proj, o_proj)
    res = run_bass_kernel_spmd(
        nc, in_maps, core_ids=list(range(N_CORES)), trace=trace
    )
    out = np.empty((B, T, D), dtype=np.float32)
    for b in range(B):
        acc = res.results[4 * b]["out"].astype(np.float32)
        for g in range(1, GROUPS):
            acc += res.results[4 * b + g]["out"].astype(np.float32)
        out[b] = acc.reshape(T, D)
    return out, res


def kernel(x, q_proj, k_proj, v_proj, o_proj, hq=None, hk=None, **_unused):
    x = np.asarray(x, dtype=np.float32)
    q_proj = np.asarray(q_proj, dtype=np.float32)
    k_proj = np.asarray(k_proj, dtype=np.float32)
    v_proj = np.asarray(v_proj, dtype=np.float32)
    o_proj = np.asarray(o_proj, dtype=np.float32)
    assert x.shape == (B, T, D), x.shape
    trace = bool(os.environ.get("KERNEL_TRACE"))
    out, _ = run(x, q_proj, k_proj, v_proj, o_proj, trace=trace)
    return out


# revision 17
# speedup vs baseline: 1.0324x; 1.0324x over previous
"""GQA self-attention block (q/k/v proj + causal softmax attention + o proj)
on 8 trn2 NeuronCores.

Sharding: batch (2) x query-head-groups (4) -> 8 cores. Core c handles
batch b=c//4 and heads [8g, 8g+8) where g=c%4 (kv heads [2g, 2g+2)).
Each core computes a partial output [T, D] = ctx_heads @ o_proj_cols.T;
the host sums the 4 partials per batch (all-reduce done host-side).

v4: all matmul operands bf16 (fp32 PSUM accumulation); every stationary
is 128 columns so fast-weight-load keeps weight loads overlapped with
in-flight matmuls. V is projected directly into key-partition layout
(x-chunk stationary, V weights moving) - no transpose pass. All DRAM
tensors are host-packed to the exact SBUF tile layout so each DMA is
long contiguous runs (16-32KB descriptors): descriptor generation, not
bandwidth, gated the strided versions. Output is bf16, one 512KB DMA
per 128-token row chunk, upcast host-side.

4-round software pipeline over 512-token blocks; per round a:
attention for query block a (S -> exp -> PV chunk pipeline, exact
128-granular causal trim) interleaved with qkv projection of block a+1
and output projection of block a-1. ACT runs the exp stream (~145us,
the only engine with exp) plus early-round out-proj drains; DVE drains
PSUM, adds causal masks, reciprocals; gpsimd does normalize muls +
small DMAs. Inputs load on two DMA rings (SP: x + out, ACT: weights).
"""

import os
import sys
from collections import deque

sys.path.insert(0, "/opt/trn_rl_repo")

import numpy as np

import concourse.bass as bass
import concourse.tile as tile
from concourse import bacc, mybir
from concourse.bass_utils import run_bass_kernel_spmd

F32 = mybir.dt.float32
BF16 = mybir.dt.bfloat16
EXP = mybir.ActivationFunctionType.Exp

B, T, D = 2, 2048, 2048
HQ, HK = 32, 8
DH = D // HQ              # 64 head dim
N_CORES = 8
GROUPS = 4                # head groups per batch
QCOLS = D // GROUPS       # 512 q cols per core
KCOLS = (D // 4) // GROUPS  # 128 k cols per core (2 kv heads)
WCOLS = QCOLS + 2 * KCOLS   # 768
BLK = 512                 # token/query block
NBLK = T // BLK           # 4
KT = D // 128             # 16 contraction tiles
NEG = -480.0              # additive mask pre-scale (-60 after 1/8 scale)

_cache = {}


def _build():
    nc = bacc.Bacc("TRN2", target_bir_lowering=False, debug=False)

    # host-packed to SBUF tile layouts (partition-major, contiguous)
    xw_d = nc.declare_dram_parameter("xw", [128, NBLK, KT, BLK], BF16, isOutput=False)
    wq_d = nc.declare_dram_parameter("wq", [128, 6, KT, 128], BF16, isOutput=False)
    op_d = nc.declare_dram_parameter("op", [128, 4, D], BF16, isOutput=False)
    masks_d = nc.declare_dram_parameter("masks", [128, 2, 128], BF16, isOutput=False)
    out_d = nc.declare_dram_parameter("out", [16, 128, D], BF16, isOutput=True)
    rcscr_d = nc.dram_tensor("rcscratch", [16, 1024], F32)

    with tile.TileContext(nc) as tc:
        with (
            tc.tile_pool(name="pers", bufs=1) as pers,
            tc.tile_pool(name="work", bufs=2) as work,
            tc.tile_pool(name="psum", bufs=1, space="PSUM") as psum,
        ):
            # ---- persistent SBUF ----
            xsb = pers.tile([128, NBLK, KT, BLK], BF16, tag="xsb")
            wq_sb = pers.tile([128, 6, KT, 128], BF16, tag="wq")
            op_sb = pers.tile([128, 4, D], BF16, tag="op")
            masks_sb = pers.tile([128, 2, 128], BF16, tag="masks")
            qt = [pers.tile([128, T], BF16, tag=f"qt{m}", name=f"qt{m}")
                  for m in range(4)]
            kp = [pers.tile([128, T], BF16, tag=f"kp{k}", name=f"kp{k}")
                  for k in range(2)]
            # vs[kv][:, c, 0:64] = V rows for key chunk c, col 64 = ones
            # (softmax denominator), cols 65:128 zero so the PV stationary
            # is 128 wide (fast weight load).
            vs = [pers.tile([128, 16, 128], BF16, tag=f"vs{k}", name=f"vs{k}")
                  for k in range(2)]
            ctx = [pers.tile([128, T], BF16, tag=f"ctx{m}", name=f"ctx{m}")
                   for m in range(4)]

            # ---- input DMA waves: SP ring carries x, ACT ring weights ----
            nc.sync.dma_start(masks_sb, masks_d[:])
            for b in range(NBLK):
                nc.sync.dma_start(xsb[:, b], xw_d[:, b])
            nc.scalar.dma_start(wq_sb[:, 0:2], wq_d[:, 0:2])
            nc.scalar.dma_start(wq_sb[:, 2:6], wq_d[:, 2:6])
            nc.scalar.dma_start(op_sb, op_d[:])
            for kv in range(2):
                nc.gpsimd.memset(vs[kv], 0.0)
                nc.gpsimd.memset(vs[kv][:, :, 64:65], 1.0)

            # ---- phase-1 block builder (qkv projection of token block b) ----
            def p1_quanta(b):
                """Yield closures; each emits one PE work quantum."""
                bsl = slice(BLK * b, BLK * b + BLK)

                def group(m, bsl=bsl, b=b):
                    ps = psum.tile([128, BLK], F32, tag="small", bufs=2,
                                   name="p1ps")
                    for k in range(KT):
                        nc.tensor.matmul(
                            ps,
                            wq_sb[:, m, k, :],
                            xsb[:, b, k, :],
                            start=(k == 0),
                            stop=(k == KT - 1),
                        )
                    if m < 4:
                        nc.vector.tensor_copy(qt[m][:, bsl], ps)
                    else:
                        nc.vector.tensor_copy(kp[0][0:64, bsl], ps[0:64, :])
                        nc.vector.tensor_copy(kp[1][64:128, bsl], ps[64:128, :])
                        # mirror each kv head to the other partition half
                        nc.gpsimd.dma_start(kp[0][64:128, bsl], kp[0][0:64, bsl])
                        nc.gpsimd.dma_start(kp[1][0:64, bsl], kp[1][64:128, bsl])

                for m in range(5):
                    yield lambda m=m: group(m)

                # V directly in [key, vdim] layout: x chunk stationary,
                # V weight columns moving.
                def vchunk(c, b=b):
                    cq = c % 4
                    ps = psum.tile([128, 128], F32, tag="small", bufs=2,
                                   name="vps")
                    for k in range(KT):
                        nc.tensor.matmul(
                            ps,
                            xsb[:, b, k, 128 * cq : 128 * cq + 128],
                            wq_sb[:, 5, k, :],
                            start=(k == 0),
                            stop=(k == KT - 1),
                        )
                    nc.vector.tensor_copy(vs[0][:, c, 0:64], ps[:, 0:64])
                    nc.vector.tensor_copy(vs[1][:, c, 0:64], ps[:, 64:128])

                def vpair(c):
                    vchunk(c)
                    vchunk(c + 1)

                for c in range(4 * b, 4 * b + 4, 2):
                    yield lambda c=c: vpair(c)

            # ---- phase-3 block builder (output projection of block b) ----
            def p3_quanta(b, stage_eng):
                for t in range(4 * b, 4 * b + 4):
                    ostage = work.tile([128, 4, 512], BF16, tag="ostage",
                                       bufs=3, name="ostage")

                    def tile_out(t, rp, ostage=ostage):
                        tsl = slice(128 * t, 128 * t + 128)
                        ps = psum.tile([128, 512], F32, tag="small", bufs=2,
                                       name="p3ps")
                        for m in range(4):
                            nc.tensor.matmul(
                                ps,
                                ctx[m][:, tsl],
                                op_sb[:, m, 512 * rp : 512 * rp + 512],
                                start=(m == 0),
                                stop=(m == 3),
                            )
                        if stage_eng == "act":
                            nc.scalar.copy(ostage[:, rp, :], ps)
                        else:
                            nc.vector.tensor_copy(ostage[:, rp, :], ps)
                        if rp % 2 == 1:
                            nc.sync.dma_start(
                                out_d[t][:, 512 * rp - 512 : 512 * rp + 512],
                                ostage[:, rp - 1 : rp + 1, :],
                            )

                    for rp in range(4):
                        yield lambda t=t, rp=rp, ostage=ostage: tile_out(t, rp)

            # ---- prologue: qkv projection of block 0, inline ----
            for q in p1_quanta(0):
                q()

            # ---- rounds ----
            pending_norm = []

            def _normalize(cu, m, a, mul_eng, fast=False):
                qsl = slice(BLK * a, BLK * a + BLK)
                ma = m * 4 + a
                if fast:
                    rcp = work.tile([1, 1024], F32, tag="rcp1", bufs=2,
                                    name="rcp")
                    nc.vector.reciprocal(rcp, cu[64:65, :])
                    nc.gpsimd.dma_start(rcscr_d[ma : ma + 1, :], rcp)
                else:
                    den128 = work.tile([128, 8], F32, tag="d128", bufs=2,
                                       name="den128")
                    nc.gpsimd.dma_start(den128, cu[64:65, :])
                    rcp = work.tile([128, 8], F32, tag="rcp", bufs=2,
                                    name="rcp")
                    nc.vector.reciprocal(rcp, den128)
                    nc.gpsimd.dma_start(rcscr_d[ma : ma + 1, :], rcp)
                bcs = work.tile([64, 1024], F32, tag="bcs", bufs=2, name="bcs")
                nc.gpsimd.dma_start(
                    bcs, rcscr_d[ma : ma + 1, :].partition_broadcast(64)
                )
                tmpB = work.tile([64, 512], BF16, tag="tb", bufs=2, name="tmpB")
                if mul_eng == "dve":
                    nc.vector.tensor_mul(
                        ctx[m][0:64, qsl], cu[0:64, 0:512], bcs[:, 0:512]
                    )
                    nc.vector.tensor_mul(
                        tmpB, cu[0:64, 512:1024], bcs[:, 512:1024]
                    )
                else:
                    nc.gpsimd.tensor_mul(
                        ctx[m][0:64, qsl], cu[0:64, 0:512], bcs[:, 0:512]
                    )
                    nc.gpsimd.tensor_mul(
                        tmpB, cu[0:64, 512:1024], bcs[:, 512:1024]
                    )
                nc.gpsimd.dma_start(ctx[m][64:128, qsl], tmpB)

            for a in range(NBLK):
                last_round = a == NBLK - 1
                p1q = deque(p1_quanta(a + 1)) if a + 1 < NBLK else deque()
                if a == 1:
                    p3q = deque(p3_quanta(0, "act"))
                elif a == 3:
                    p3q = deque(p3_quanta(1, "dve"))
                    p3q.extend(p3_quanta(2, "dve"))
                else:
                    p3q = deque()
                nj = 4 * (a + 1)
                chunk_i = 0
                for m in range(4):
                    kv = m // 2
                    ctxAB = psum.tile([128, 1024], F32, tag="ctx", bufs=1,
                                      name="ctxAB")
                    pend = []

                    def flush_pv(n, m=m, kv=kv, ctxAB=ctxAB, pend=pend, nj=nj):
                        while len(pend) > n:
                            pE, pjc, plo = pend.pop(0)
                            for h2 in range(2):
                                nc.tensor.matmul(
                                    ctxAB[:, 512 * h2 + plo : 512 * h2 + 512],
                                    vs[kv][:, pjc, :],
                                    pE[:, h2, plo:512],
                                    start=(pjc == 0),
                                    stop=(pjc == nj - 1 and h2 == 1),
                                )

                    for jc in range(nj):
                        o = jc - 4 * a
                        lo = 128 * o if o >= 0 else 0
                        jsl = slice(128 * jc, 128 * jc + 128)
                        S = psum.tile([128, 2, 512], F32, tag="S", bufs=2,
                                      name="S")
                        for h2 in range(2):
                            nc.tensor.matmul(
                                S[:, h2, lo:512],
                                kp[kv][64 * h2 : 64 * h2 + 64, jsl],
                                qt[m][64 * h2 : 64 * h2 + 64,
                                      BLK * a + lo : BLK * a + 512],
                                start=True,
                                stop=True,
                                tile_position=(64 * h2, 0),
                            )
                        if o >= 0:
                            nc.vector.tensor_add(
                                S[:, :, lo : lo + 128],
                                S[:, :, lo : lo + 128],
                                masks_sb,
                            )
                        E = work.tile([128, 2, 512], BF16, tag="E", bufs=6,
                                      name="E")
                        nc.scalar.activation(
                            E[:, :, lo:512], S[:, :, lo:512], EXP, scale=0.125
                        )
                        pend.append((E, jc, lo))
                        flush_pv(2)
                        # interleave filler PE work (next-block qkv proj,
                        # prev-block out proj)
                        if p3q:
                            p3q.popleft()()
                        if p1q and chunk_i % 2 == 1:
                            p1q.popleft()()
                        chunk_i += 1

                    flush_pv(0)

                    # fast PSUM release; normalize runs deferred except in
                    # the last round (the tail's out-proj waits on it)
                    cu = work.tile([65, 1024], F32, tag="cu", bufs=3, name="cu")
                    nc.vector.tensor_copy(cu, ctxAB[0:65, :])
                    if p3q:
                        p3q.popleft()()
                    if p1q:
                        p1q.popleft()()
                    if last_round:
                        _normalize(cu, m, a, "dve", fast=(m == 3))
                    else:
                        pending_norm.append(
                            lambda cu=cu, m=m, a=a: _normalize(cu, m, a, "gps")
                        )
                        while len(pending_norm) > 2:
                            pending_norm.pop(0)()

                # round end: flush fillers that later rounds depend on
                while p1q:
                    p1q.popleft()()
                while p3q:
                    p3q.popleft()()
                while pending_norm:
                    pending_norm.pop(0)()

            # ---- tail: output projection of block 3 ----
            for q in p3_quanta(NBLK - 1, "act"):
                q()

    nc.compile()
    return nc


def _host_inputs(x, q_proj, k_proj, v_proj, o_proj):
    """Per-core input dicts (numpy, bf16, packed to SBUF tile layouts)."""
    import ml_dtypes

    bf16 = ml_dtypes.bfloat16
    masks = np.zeros((128, 2, 128), dtype=np.float32)
    jj = np.arange(128)[:, None]
    ii = np.arange(128)[None, :]
    tri = np.where(jj <= ii, 0.0, NEG)
    masks[:, 0, :] = tri
    masks[:, 1, :] = tri
    masks = masks.astype(bf16)

    # x[b].T is [D, T]; pack to [128p, NBLK, KT, BLK]
    xw = []
    for b in range(B):
        xT = np.ascontiguousarray(x[b].T).astype(bf16)
        xw.append(
            np.ascontiguousarray(
                xT.reshape(KT, 128, NBLK, BLK).transpose(1, 2, 0, 3)
            )
        )
    in_maps = []
    for c in range(N_CORES):
        b, g = divmod(c, GROUPS)
        wqkv = np.concatenate(
            [
                q_proj[QCOLS * g : QCOLS * g + QCOLS].T,
                k_proj[KCOLS * g : KCOLS * g + KCOLS].T,
                v_proj[KCOLS * g : KCOLS * g + KCOLS].T,
            ],
            axis=1,
        ).astype(bf16)  # [D, WCOLS]
        # pack to [128p, 6m, KT, 128]
        wq = np.ascontiguousarray(
            wqkv.reshape(KT, 128, 6, 128).transpose(1, 2, 0, 3)
        )
        opj = np.ascontiguousarray(
            o_proj[:, QCOLS * g : QCOLS * g + QCOLS].T
        ).astype(bf16)  # [QCOLS, D]
        op = np.ascontiguousarray(opj.reshape(4, 128, D).transpose(1, 0, 2))
        in_maps.append(
            {"xw": xw[b], "wq": wq, "op": op, "masks": masks}
        )
    return in_maps


def run(x, q_proj, k_proj, v_proj, o_proj, trace=False):
    """Run on hardware; returns (output [B,T,D] f32, BassKernelResults)."""
    if "nc" not in _cache:
        _cache["nc"] = _build()
    nc = _cache["nc"]
    in_maps = _host_inputs(x, q_proj, k_proj, v_proj, o_proj)
    res = run_bass_kernel_spmd(
        nc, in_maps, core_ids=list(range(N_CORES)), trace=trace
    )
    out = np.empty((B, T, D), dtype=np.float32)
    for b in range(B):
        acc = res.results[4 * b]["out"].astype(np.float32)
        for g in range(1, GROUPS):
            acc += res.results[4 * b + g]["out"].astype(np.float32)
        out[b] = acc.reshape(T, D)
    return out, res


def kernel(x, q_proj, k_proj, v_proj, o_proj, hq=None, hk=None, **_unused):
    x = np.asarray(x, dtype=np.float32)
    q_proj = np.asarray(q_proj, dtype=np.float32)
    k_proj = np.asarray(k_proj, dtype=np.float32)
    v_proj = np.asarray(v_proj, dtype=np.float32)
    o_proj = np.asarray(o_proj, dtype=np.float32)
    assert x.shape == (B, T, D), x.shape
    trace = bool(os.environ.get("KERNEL_TRACE"))
    out, _ = run(x, q_proj, k_proj, v_proj, o_proj, trace=trace)
    return out


# revision 18
# speedup vs baseline: 1.0497x; 1.0168x over previous
"""GQA self-attention block (q/k/v proj + causal softmax attention + o proj)
on 8 trn2 NeuronCores.

Sharding: batch (2) x query-head-groups (4) -> 8 cores. Core c handles
batch b=c//4 and heads [8g, 8g+8) where g=c%4 (kv heads [2g, 2g+2)).
Each core computes a partial output [T, D] = ctx_heads @ o_proj_cols.T;
the host sums the 4 partials per batch (all-reduce done host-side).

v4: all matmul operands bf16 (fp32 PSUM accumulation); every stationary
is 128 columns so fast-weight-load keeps weight loads overlapped with
in-flight matmuls. V is projected directly into key-partition layout
(x-chunk stationary, V weights moving) - no transpose pass. All DRAM
tensors are host-packed to the exact SBUF tile layout so each DMA is
long contiguous runs (16-32KB descriptors): descriptor generation, not
bandwidth, gated the strided versions. Output is bf16, one 512KB DMA
per 128-token row chunk, upcast host-side.

4-round software pipeline over 512-token blocks; per round a:
attention for query block a (S -> exp -> PV chunk pipeline, exact
128-granular causal trim) interleaved with qkv projection of block a+1
and output projection of block a-1. ACT runs the exp stream (~145us,
the only engine with exp) plus early-round out-proj drains; DVE drains
PSUM, adds causal masks, reciprocals; gpsimd does normalize muls +
small DMAs. Inputs load on two DMA rings (SP: x + out, ACT: weights).
"""

import os
import sys
from collections import deque

sys.path.insert(0, "/opt/trn_rl_repo")

import numpy as np

import concourse.bass as bass
import concourse.tile as tile
from concourse import bacc, mybir
from concourse.bass_utils import run_bass_kernel_spmd

F32 = mybir.dt.float32
BF16 = mybir.dt.bfloat16
EXP = mybir.ActivationFunctionType.Exp

B, T, D = 2, 2048, 2048
HQ, HK = 32, 8
DH = D // HQ              # 64 head dim
N_CORES = 8
GROUPS = 4                # head groups per batch
QCOLS = D // GROUPS       # 512 q cols per core
KCOLS = (D // 4) // GROUPS  # 128 k cols per core (2 kv heads)
WCOLS = QCOLS + 2 * KCOLS   # 768
BLK = 512                 # token/query block
NBLK = T // BLK           # 4
KT = D // 128             # 16 contraction tiles
NEG = -480.0              # additive mask pre-scale (-60 after 1/8 scale)

_cache = {}


def _build():
    nc = bacc.Bacc("TRN2", target_bir_lowering=False, debug=False)

    # host-packed to SBUF tile layouts (partition-major, contiguous)
    xw_d = nc.declare_dram_parameter("xw", [128, NBLK, KT, BLK], BF16, isOutput=False)
    wq_d = nc.declare_dram_parameter("wq", [128, 6, KT, 128], BF16, isOutput=False)
    op_d = nc.declare_dram_parameter("op", [128, 4, D], BF16, isOutput=False)
    masks_d = nc.declare_dram_parameter("masks", [128, 2, 128], BF16, isOutput=False)
    out_d = nc.declare_dram_parameter("out", [16, 128, D], BF16, isOutput=True)
    rcscr_d = nc.dram_tensor("rcscratch", [16, 1024], F32)

    with tile.TileContext(nc) as tc:
        with (
            tc.tile_pool(name="pers", bufs=1) as pers,
            tc.tile_pool(name="work", bufs=2) as work,
            tc.tile_pool(name="psum", bufs=1, space="PSUM") as psum,
        ):
            # ---- persistent SBUF ----
            xsb = pers.tile([128, NBLK, KT, BLK], BF16, tag="xsb")
            wq_sb = pers.tile([128, 6, KT, 128], BF16, tag="wq")
            op_sb = pers.tile([128, 4, D], BF16, tag="op")
            masks_sb = pers.tile([128, 2, 128], BF16, tag="masks")
            qt = [pers.tile([128, T], BF16, tag=f"qt{m}", name=f"qt{m}")
                  for m in range(4)]
            kp = [pers.tile([128, T], BF16, tag=f"kp{k}", name=f"kp{k}")
                  for k in range(2)]
            # vs[kv][:, c, 0:64] = V rows for key chunk c, col 64 = ones
            # (softmax denominator), cols 65:128 zero so the PV stationary
            # is 128 wide (fast weight load).
            vs = [pers.tile([128, 16, 128], BF16, tag=f"vs{k}", name=f"vs{k}")
                  for k in range(2)]
            ctx = [pers.tile([128, T], BF16, tag=f"ctx{m}", name=f"ctx{m}")
                   for m in range(4)]

            # ---- input DMA waves: SP ring carries x, ACT ring weights ----
            nc.sync.dma_start(masks_sb, masks_d[:])
            for b in range(NBLK):
                nc.sync.dma_start(xsb[:, b], xw_d[:, b])
            nc.scalar.dma_start(wq_sb[:, 0:2], wq_d[:, 0:2])
            nc.scalar.dma_start(wq_sb[:, 2:6], wq_d[:, 2:6])
            nc.scalar.dma_start(op_sb, op_d[:])
            for kv in range(2):
                nc.gpsimd.memset(vs[kv], 0.0)
                nc.gpsimd.memset(vs[kv][:, :, 64:65], 1.0)

            # ---- phase-1 block builder (qkv projection of token block b) ----
            def p1_quanta(b):
                """Yield closures; each emits one PE work quantum."""
                bsl = slice(BLK * b, BLK * b + BLK)

                def group(m, bsl=bsl, b=b):
                    ps = psum.tile([128, BLK], F32, tag="small", bufs=2,
                                   name="p1ps")
                    for k in range(KT):
                        nc.tensor.matmul(
                            ps,
                            wq_sb[:, m, k, :],
                            xsb[:, b, k, :],
                            start=(k == 0),
                            stop=(k == KT - 1),
                        )
                    if m < 4:
                        nc.vector.tensor_copy(qt[m][:, bsl], ps)
                    else:
                        nc.vector.tensor_copy(kp[0][0:64, bsl], ps[0:64, :])
                        nc.vector.tensor_copy(kp[1][64:128, bsl], ps[64:128, :])
                        # mirror each kv head to the other partition half
                        nc.gpsimd.dma_start(kp[0][64:128, bsl], kp[0][0:64, bsl])
                        nc.gpsimd.dma_start(kp[1][0:64, bsl], kp[1][64:128, bsl])

                for m in range(5):
                    yield lambda m=m: group(m)

                # V directly in [key, vdim] layout: x chunk stationary,
                # V weight columns moving.
                def vchunk(c, b=b):
                    cq = c % 4
                    ps = psum.tile([128, 128], F32, tag="small", bufs=2,
                                   name="vps")
                    for k in range(KT):
                        nc.tensor.matmul(
                            ps,
                            xsb[:, b, k, 128 * cq : 128 * cq + 128],
                            wq_sb[:, 5, k, :],
                            start=(k == 0),
                            stop=(k == KT - 1),
                        )
                    nc.vector.tensor_copy(vs[0][:, c, 0:64], ps[:, 0:64])
                    nc.vector.tensor_copy(vs[1][:, c, 0:64], ps[:, 64:128])

                def vpair(c):
                    vchunk(c)
                    vchunk(c + 1)

                for c in range(4 * b, 4 * b + 4, 2):
                    yield lambda c=c: vpair(c)

            # ---- phase-3 block builder (output projection of block b) ----
            def p3_quanta(b, stage_eng):
                for t in range(4 * b, 4 * b + 4):
                    ostage = work.tile([128, 4, 512], BF16, tag="ostage",
                                       bufs=3, name="ostage")

                    def tile_out(t, rp, ostage=ostage):
                        tsl = slice(128 * t, 128 * t + 128)
                        ps = psum.tile([128, 512], F32, tag="small", bufs=2,
                                       name="p3ps")
                        for m in range(4):
                            nc.tensor.matmul(
                                ps,
                                ctx[m][:, tsl],
                                op_sb[:, m, 512 * rp : 512 * rp + 512],
                                start=(m == 0),
                                stop=(m == 3),
                            )
                        if stage_eng == "act":
                            nc.scalar.copy(ostage[:, rp, :], ps)
                        else:
                            nc.vector.tensor_copy(ostage[:, rp, :], ps)
                        if rp % 2 == 1:
                            nc.sync.dma_start(
                                out_d[t][:, 512 * rp - 512 : 512 * rp + 512],
                                ostage[:, rp - 1 : rp + 1, :],
                            )

                    for rp in range(4):
                        yield lambda t=t, rp=rp, ostage=ostage: tile_out(t, rp)

            # ---- prologue: qkv projection of block 0, inline ----
            for q in p1_quanta(0):
                q()

            # ---- rounds ----
            pending_norm = []

            def _normalize(cu, m, a, mul_eng, fast=False):
                qsl = slice(BLK * a, BLK * a + BLK)
                ma = m * 4 + a
                if fast:
                    rcp = work.tile([1, 1024], F32, tag="rcp1", bufs=2,
                                    name="rcp")
                    nc.vector.reciprocal(rcp, cu[64:65, :])
                    nc.gpsimd.dma_start(rcscr_d[ma : ma + 1, :], rcp)
                else:
                    den128 = work.tile([128, 8], F32, tag="d128", bufs=2,
                                       name="den128")
                    nc.gpsimd.dma_start(den128, cu[64:65, :])
                    rcp = work.tile([128, 8], F32, tag="rcp", bufs=2,
                                    name="rcp")
                    nc.vector.reciprocal(rcp, den128)
                    nc.gpsimd.dma_start(rcscr_d[ma : ma + 1, :], rcp)
                bcs = work.tile([64, 1024], F32, tag="bcs", bufs=2, name="bcs")
                nc.gpsimd.dma_start(
                    bcs, rcscr_d[ma : ma + 1, :].partition_broadcast(64)
                )
                tmpB = work.tile([64, 512], BF16, tag="tb", bufs=2, name="tmpB")
                if mul_eng == "dve":
                    nc.vector.tensor_mul(
                        ctx[m][0:64, qsl], cu[0:64, 0:512], bcs[:, 0:512]
                    )
                    nc.vector.tensor_mul(
                        tmpB, cu[0:64, 512:1024], bcs[:, 512:1024]
                    )
                else:
                    nc.gpsimd.tensor_mul(
                        ctx[m][0:64, qsl], cu[0:64, 0:512], bcs[:, 0:512]
                    )
                    nc.gpsimd.tensor_mul(
                        tmpB, cu[0:64, 512:1024], bcs[:, 512:1024]
                    )
                nc.gpsimd.dma_start(ctx[m][64:128, qsl], tmpB)

            for a in range(NBLK):
                last_round = a == NBLK - 1
                p1q = deque(p1_quanta(a + 1)) if a + 1 < NBLK else deque()
                if a == 1:
                    p3q = deque(p3_quanta(0, "act"))
                elif a == 3:
                    p3q = deque(p3_quanta(1, "dve"))
                    p3q.extend(p3_quanta(2, "dve"))
                else:
                    p3q = deque()
                nj = 4 * (a + 1)
                chunk_i = 0
                for m in range(4):
                    kv = m // 2
                    ctxAB = psum.tile([128, 1024], F32, tag="ctx", bufs=1,
                                      name="ctxAB")
                    pend = []

                    def flush_pv(n, m=m, kv=kv, ctxAB=ctxAB, pend=pend, nj=nj):
                        while len(pend) > n:
                            pE, pjc, plo = pend.pop(0)
                            for h2 in range(2):
                                nc.tensor.matmul(
                                    ctxAB[:, 512 * h2 + plo : 512 * h2 + 512],
                                    vs[kv][:, pjc, :],
                                    pE[:, h2, plo:512],
                                    start=(pjc == 0),
                                    stop=(pjc == nj - 1 and h2 == 1),
                                )

                    for jc in range(nj):
                        o = jc - 4 * a
                        lo = 128 * o if o >= 0 else 0
                        jsl = slice(128 * jc, 128 * jc + 128)
                        S = psum.tile([128, 2, 512], F32, tag="S", bufs=2,
                                      name="S")
                        for h2 in range(2):
                            nc.tensor.matmul(
                                S[:, h2, lo:512],
                                kp[kv][64 * h2 : 64 * h2 + 64, jsl],
                                qt[m][64 * h2 : 64 * h2 + 64,
                                      BLK * a + lo : BLK * a + 512],
                                start=True,
                                stop=True,
                                tile_position=(64 * h2, 0),
                            )
                        if o >= 0:
                            nc.vector.tensor_add(
                                S[:, :, lo : lo + 128],
                                S[:, :, lo : lo + 128],
                                masks_sb,
                            )
                        E = work.tile([128, 2, 512], BF16, tag="E", bufs=6,
                                      name="E")
                        nc.scalar.activation(
                            E[:, :, lo:512], S[:, :, lo:512], EXP, scale=0.125
                        )
                        pend.append((E, jc, lo))
                        flush_pv(2)
                        # interleave filler PE work (next-block qkv proj,
                        # prev-block out proj); in the last round hold back
                        # a few quanta to cover the final normalize chain
                        if p3q and (not last_round or len(p3q) > 8):
                            p3q.popleft()()
                        if p1q and chunk_i % 2 == 1:
                            p1q.popleft()()
                        chunk_i += 1

                    flush_pv(0)

                    # fast PSUM release; normalize runs deferred except in
                    # the last round (the tail's out-proj waits on it)
                    cu = work.tile([65, 1024], F32, tag="cu", bufs=3, name="cu")
                    nc.vector.tensor_copy(cu, ctxAB[0:65, :])
                    if p3q and (not last_round or len(p3q) > 8):
                        p3q.popleft()()
                    if p1q:
                        p1q.popleft()()
                    if last_round:
                        _normalize(cu, m, a, "dve", fast=(m == 3))
                    else:
                        pending_norm.append(
                            lambda cu=cu, m=m, a=a: _normalize(cu, m, a, "gps")
                        )
                        while len(pending_norm) > 2:
                            pending_norm.pop(0)()

                # round end: flush fillers that later rounds depend on
                while p1q:
                    p1q.popleft()()
                while p3q:
                    p3q.popleft()()
                while pending_norm:
                    pending_norm.pop(0)()

            # ---- tail: output projection of block 3 ----
            for q in p3_quanta(NBLK - 1, "act"):
                q()

    nc.compile()
    return nc


def _host_inputs(x, q_proj, k_proj, v_proj, o_proj):
    """Per-core input dicts (numpy, bf16, packed to SBUF tile layouts)."""
    import ml_dtypes

    bf16 = ml_dtypes.bfloat16
    masks = np.zeros((128, 2, 128), dtype=np.float32)
    jj = np.arange(128)[:, None]
    ii = np.arange(128)[None, :]
    tri = np.where(jj <= ii, 0.0, NEG)
    masks[:, 0, :] = tri
    masks[:, 1, :] = tri
    masks = masks.astype(bf16)

    # x[b].T is [D, T]; pack to [128p, NBLK, KT, BLK]
    xw = []
    for b in range(B):
        xT = np.ascontiguousarray(x[b].T).astype(bf16)
        xw.append(
            np.ascontiguousarray(
                xT.reshape(KT, 128, NBLK, BLK).transpose(1, 2, 0, 3)
            )
        )
    in_maps = []
    for c in range(N_CORES):
        b, g = divmod(c, GROUPS)
        wqkv = np.concatenate(
            [
                q_proj[QCOLS * g : QCOLS * g + QCOLS].T,
                k_proj[KCOLS * g : KCOLS * g + KCOLS].T,
                v_proj[KCOLS * g : KCOLS * g + KCOLS].T,
            ],
            axis=1,
        ).astype(bf16)  # [D, WCOLS]
        # pack to [128p, 6m, KT, 128]
        wq = np.ascontiguousarray(
            wqkv.reshape(KT, 128, 6, 128).transpose(1, 2, 0, 3)
        )
        opj = np.ascontiguousarray(
            o_proj[:, QCOLS * g : QCOLS * g + QCOLS].T
        ).astype(bf16)  # [QCOLS, D]
        op = np.ascontiguousarray(opj.reshape(4, 128, D).transpose(1, 0, 2))
        in_maps.append(
            {"xw": xw[b], "wq": wq, "op": op, "masks": masks}
        )
    return in_maps


def run(x, q_proj, k_proj, v_proj, o_proj, trace=False):
    """Run on hardware; returns (output [B,T,D] f32, BassKernelResults)."""
    if "nc" not in _cache:
        _cache["nc"] = _build()
    nc = _cache["nc"]
    in_maps = _host_inputs(x, q_proj, k_proj, v_proj, o_proj)
    res = run_bass_kernel_spmd(
        nc, in_maps, core_ids=list(range(N_CORES)), trace=trace
    )
    out = np.empty((B, T, D), dtype=np.float32)
    for b in range(B):
        acc = res.results[4 * b]["out"].astype(np.float32)
        for g in range(1, GROUPS):
            acc += res.results[4 * b + g]["out"].astype(np.float32)
        out[b] = acc.reshape(T, D)
    return out, res


def kernel(x, q_proj, k_proj, v_proj, o_proj, hq=None, hk=None, **_unused):
    x = np.asarray(x, dtype=np.float32)
    q_proj = np.asarray(q_proj, dtype=np.float32)
    k_proj = np.asarray(k_proj, dtype=np.float32)
    v_proj = np.asarray(v_proj, dtype=np.float32)
    o_proj = np.asarray(o_proj, dtype=np.float32)
    assert x.shape == (B, T, D), x.shape
    trace = bool(os.environ.get("KERNEL_TRACE"))
    out, _ = run(x, q_proj, k_proj, v_proj, o_proj, trace=trace)
    return out


# revision 19
# speedup vs baseline: 1.0515x; 1.0016x over previous
"""GQA self-attention block (q/k/v proj + causal softmax attention + o proj)
on 8 trn2 NeuronCores.

Sharding: batch (2) x query-head-groups (4) -> 8 cores. Core c handles
batch b=c//4 and heads [8g, 8g+8) where g=c%4 (kv heads [2g, 2g+2)).
Each core computes a partial output [T, D] = ctx_heads @ o_proj_cols.T;
the host sums the 4 partials per batch (all-reduce done host-side).

v4: all matmul operands bf16 (fp32 PSUM accumulation); every stationary
is 128 columns so fast-weight-load keeps weight loads overlapped with
in-flight matmuls. V is projected directly into key-partition layout
(x-chunk stationary, V weights moving) - no transpose pass. All DRAM
tensors are host-packed to the exact SBUF tile layout so each DMA is
long contiguous runs (16-32KB descriptors): descriptor generation, not
bandwidth, gated the strided versions. Output is bf16, one 512KB DMA
per 128-token row chunk, upcast host-side.

4-round software pipeline over 512-token blocks; per round a:
attention for query block a (S -> exp -> PV chunk pipeline, exact
128-granular causal trim) interleaved with qkv projection of block a+1
and output projection of block a-1. ACT runs the exp stream (~145us,
the only engine with exp) plus early-round out-proj drains; DVE drains
PSUM, adds causal masks, reciprocals; gpsimd does normalize muls +
small DMAs. Inputs load on two DMA rings (SP: x + out, ACT: weights).
"""

import os
import sys
from collections import deque

sys.path.insert(0, "/opt/trn_rl_repo")

import numpy as np

import concourse.bass as bass
import concourse.tile as tile
from concourse import bacc, mybir
from concourse.bass_utils import run_bass_kernel_spmd

F32 = mybir.dt.float32
BF16 = mybir.dt.bfloat16
EXP = mybir.ActivationFunctionType.Exp

B, T, D = 2, 2048, 2048
HQ, HK = 32, 8
DH = D // HQ              # 64 head dim
N_CORES = 8
GROUPS = 4                # head groups per batch
QCOLS = D // GROUPS       # 512 q cols per core
KCOLS = (D // 4) // GROUPS  # 128 k cols per core (2 kv heads)
WCOLS = QCOLS + 2 * KCOLS   # 768
BLK = 512                 # token/query block
NBLK = T // BLK           # 4
KT = D // 128             # 16 contraction tiles
NEG = -480.0              # additive mask pre-scale (-60 after 1/8 scale)

_cache = {}


def _build():
    nc = bacc.Bacc("TRN2", target_bir_lowering=False, debug=False)

    # host-packed to SBUF tile layouts (partition-major, contiguous)
    xw_d = nc.declare_dram_parameter("xw", [128, NBLK, KT, BLK], BF16, isOutput=False)
    wq_d = nc.declare_dram_parameter("wq", [128, 6, KT, 128], BF16, isOutput=False)
    op_d = nc.declare_dram_parameter("op", [128, 4, D], BF16, isOutput=False)
    masks_d = nc.declare_dram_parameter("masks", [128, 2, 128], BF16, isOutput=False)
    out_d = nc.declare_dram_parameter("out", [16, 128, D], BF16, isOutput=True)
    rcscr_d = nc.dram_tensor("rcscratch", [16, 1024], F32)

    with tile.TileContext(nc) as tc:
        with (
            tc.tile_pool(name="pers", bufs=1) as pers,
            tc.tile_pool(name="work", bufs=2) as work,
            tc.tile_pool(name="psum", bufs=1, space="PSUM") as psum,
        ):
            # ---- persistent SBUF ----
            xsb = pers.tile([128, NBLK, KT, BLK], BF16, tag="xsb")
            wq_sb = pers.tile([128, 6, KT, 128], BF16, tag="wq")
            op_sb = pers.tile([128, 4, D], BF16, tag="op")
            masks_sb = pers.tile([128, 2, 128], BF16, tag="masks")
            qt = [pers.tile([128, T], BF16, tag=f"qt{m}", name=f"qt{m}")
                  for m in range(4)]
            kp = [pers.tile([128, T], BF16, tag=f"kp{k}", name=f"kp{k}")
                  for k in range(2)]
            # vs[kv][:, c, 0:64] = V rows for key chunk c, col 64 = ones
            # (softmax denominator), cols 65:128 zero so the PV stationary
            # is 128 wide (fast weight load).
            vs = [pers.tile([128, 16, 128], BF16, tag=f"vs{k}", name=f"vs{k}")
                  for k in range(2)]
            ctx = [pers.tile([128, T], BF16, tag=f"ctx{m}", name=f"ctx{m}")
                   for m in range(4)]

            # ---- input DMA waves: SP ring carries x, ACT ring weights ----
            nc.sync.dma_start(masks_sb, masks_d[:])
            for b in range(NBLK):
                nc.sync.dma_start(xsb[:, b], xw_d[:, b])
            nc.scalar.dma_start(wq_sb[:, 0:2], wq_d[:, 0:2])
            nc.scalar.dma_start(wq_sb[:, 2:6], wq_d[:, 2:6])
            nc.scalar.dma_start(op_sb, op_d[:])
            for kv in range(2):
                nc.gpsimd.memset(vs[kv], 0.0)
                nc.gpsimd.memset(vs[kv][:, :, 64:65], 1.0)

            # ---- phase-1 block builder (qkv projection of token block b) ----
            def p1_quanta(b):
                """Yield closures; each emits one PE work quantum."""
                bsl = slice(BLK * b, BLK * b + BLK)

                def group(m, bsl=bsl, b=b):
                    ps = psum.tile([128, BLK], F32, tag="small", bufs=2,
                                   name="p1ps")
                    for k in range(KT):
                        nc.tensor.matmul(
                            ps,
                            wq_sb[:, m, k, :],
                            xsb[:, b, k, :],
                            start=(k == 0),
                            stop=(k == KT - 1),
                        )
                    if m < 4:
                        nc.vector.tensor_copy(qt[m][:, bsl], ps)
                    else:
                        nc.vector.tensor_copy(kp[0][0:64, bsl], ps[0:64, :])
                        nc.vector.tensor_copy(kp[1][64:128, bsl], ps[64:128, :])
                        # mirror each kv head to the other partition half
                        nc.gpsimd.dma_start(kp[0][64:128, bsl], kp[0][0:64, bsl])
                        nc.gpsimd.dma_start(kp[1][0:64, bsl], kp[1][64:128, bsl])

                for m in range(5):
                    yield lambda m=m: group(m)

                # V directly in [key, vdim] layout: x chunk stationary,
                # V weight columns moving.
                def vchunk(c, b=b):
                    cq = c % 4
                    ps = psum.tile([128, 128], F32, tag="small", bufs=2,
                                   name="vps")
                    for k in range(KT):
                        nc.tensor.matmul(
                            ps,
                            xsb[:, b, k, 128 * cq : 128 * cq + 128],
                            wq_sb[:, 5, k, :],
                            start=(k == 0),
                            stop=(k == KT - 1),
                        )
                    nc.vector.tensor_copy(vs[0][:, c, 0:64], ps[:, 0:64])
                    nc.vector.tensor_copy(vs[1][:, c, 0:64], ps[:, 64:128])

                def vpair(c):
                    vchunk(c)
                    vchunk(c + 1)

                for c in range(4 * b, 4 * b + 4, 2):
                    yield lambda c=c: vpair(c)

            # ---- phase-3 block builder (output projection of block b) ----
            def p3_quanta(b, stage_eng):
                for t in range(4 * b, 4 * b + 4):
                    ostage = work.tile([128, 4, 512], BF16, tag="ostage",
                                       bufs=3, name="ostage")

                    def tile_out(t, rp, ostage=ostage):
                        tsl = slice(128 * t, 128 * t + 128)
                        ps = psum.tile([128, 512], F32, tag="small", bufs=2,
                                       name="p3ps")
                        for m in range(4):
                            nc.tensor.matmul(
                                ps,
                                ctx[m][:, tsl],
                                op_sb[:, m, 512 * rp : 512 * rp + 512],
                                start=(m == 0),
                                stop=(m == 3),
                            )
                        if stage_eng == "act":
                            nc.scalar.copy(ostage[:, rp, :], ps)
                        else:
                            nc.vector.tensor_copy(ostage[:, rp, :], ps)
                        if rp % 2 == 1:
                            nc.sync.dma_start(
                                out_d[t][:, 512 * rp - 512 : 512 * rp + 512],
                                ostage[:, rp - 1 : rp + 1, :],
                            )

                    for rp in range(4):
                        yield lambda t=t, rp=rp, ostage=ostage: tile_out(t, rp)

            # ---- prologue: qkv projection of block 0, inline ----
            for q in p1_quanta(0):
                q()

            # ---- rounds ----
            pending_norm = []

            def _normalize(cu, m, a, mul_eng, fast=False):
                qsl = slice(BLK * a, BLK * a + BLK)
                ma = m * 4 + a
                if fast:
                    rcp = work.tile([1, 1024], F32, tag="rcp1", bufs=2,
                                    name="rcp")
                    nc.vector.reciprocal(rcp, cu[64:65, :])
                    nc.sync.dma_start(rcscr_d[ma : ma + 1, :], rcp)
                else:
                    den128 = work.tile([128, 8], F32, tag="d128", bufs=2,
                                       name="den128")
                    nc.gpsimd.dma_start(den128, cu[64:65, :])
                    rcp = work.tile([128, 8], F32, tag="rcp", bufs=2,
                                    name="rcp")
                    nc.vector.reciprocal(rcp, den128)
                    nc.gpsimd.dma_start(rcscr_d[ma : ma + 1, :], rcp)
                bcs = work.tile([64, 1024], F32, tag="bcs", bufs=2, name="bcs")
                (nc.sync if fast else nc.gpsimd).dma_start(
                    bcs, rcscr_d[ma : ma + 1, :].partition_broadcast(64)
                )
                tmpB = work.tile([64, 512], BF16, tag="tb", bufs=2, name="tmpB")
                if mul_eng == "dve":
                    nc.vector.tensor_mul(
                        ctx[m][0:64, qsl], cu[0:64, 0:512], bcs[:, 0:512]
                    )
                    nc.vector.tensor_mul(
                        tmpB, cu[0:64, 512:1024], bcs[:, 512:1024]
                    )
                else:
                    nc.gpsimd.tensor_mul(
                        ctx[m][0:64, qsl], cu[0:64, 0:512], bcs[:, 0:512]
                    )
                    nc.gpsimd.tensor_mul(
                        tmpB, cu[0:64, 512:1024], bcs[:, 512:1024]
                    )
                (nc.sync if fast else nc.gpsimd).dma_start(
                    ctx[m][64:128, qsl], tmpB
                )

            for a in range(NBLK):
                last_round = a == NBLK - 1
                p1q = deque(p1_quanta(a + 1)) if a + 1 < NBLK else deque()
                if a == 1:
                    p3q = deque(p3_quanta(0, "act"))
                elif a == 3:
                    p3q = deque(p3_quanta(1, "dve"))
                    p3q.extend(p3_quanta(2, "dve"))
                else:
                    p3q = deque()
                nj = 4 * (a + 1)
                chunk_i = 0
                for m in range(4):
                    kv = m // 2
                    ctxAB = psum.tile([128, 1024], F32, tag="ctx", bufs=1,
                                      name="ctxAB")
                    pend = []

                    def flush_pv(n, m=m, kv=kv, ctxAB=ctxAB, pend=pend, nj=nj):
                        while len(pend) > n:
                            pE, pjc, plo = pend.pop(0)
                            for h2 in range(2):
                                nc.tensor.matmul(
                                    ctxAB[:, 512 * h2 + plo : 512 * h2 + 512],
                                    vs[kv][:, pjc, :],
                                    pE[:, h2, plo:512],
                                    start=(pjc == 0),
                                    stop=(pjc == nj - 1 and h2 == 1),
                                )

                    for jc in range(nj):
                        o = jc - 4 * a
                        lo = 128 * o if o >= 0 else 0
                        jsl = slice(128 * jc, 128 * jc + 128)
                        S = psum.tile([128, 2, 512], F32, tag="S", bufs=2,
                                      name="S")
                        for h2 in range(2):
                            nc.tensor.matmul(
                                S[:, h2, lo:512],
                                kp[kv][64 * h2 : 64 * h2 + 64, jsl],
                                qt[m][64 * h2 : 64 * h2 + 64,
                                      BLK * a + lo : BLK * a + 512],
                                start=True,
                                stop=True,
                                tile_position=(64 * h2, 0),
                            )
                        if o >= 0:
                            nc.vector.tensor_add(
                                S[:, :, lo : lo + 128],
                                S[:, :, lo : lo + 128],
                                masks_sb,
                            )
                        E = work.tile([128, 2, 512], BF16, tag="E", bufs=6,
                                      name="E")
                        nc.scalar.activation(
                            E[:, :, lo:512], S[:, :, lo:512], EXP, scale=0.125
                        )
                        pend.append((E, jc, lo))
                        flush_pv(2)
                        # interleave filler PE work (next-block qkv proj,
                        # prev-block out proj); in the last round hold back
                        # a few quanta to cover the final normalize chain
                        if p3q and (not last_round or len(p3q) > 16):
                            p3q.popleft()()
                        if p1q and chunk_i % 2 == 1:
                            p1q.popleft()()
                        chunk_i += 1

                    flush_pv(0)

                    # fast PSUM release; normalize runs deferred except in
                    # the last round (the tail's out-proj waits on it)
                    cu = work.tile([65, 1024], F32, tag="cu", bufs=3, name="cu")
                    nc.vector.tensor_copy(cu, ctxAB[0:65, :])
                    if p3q and (not last_round or len(p3q) > 16):
                        p3q.popleft()()
                    if p1q:
                        p1q.popleft()()
                    if last_round:
                        _normalize(cu, m, a, "dve", fast=(m == 3))
                    else:
                        pending_norm.append(
                            lambda cu=cu, m=m, a=a: _normalize(cu, m, a, "gps")
                        )
                        while len(pending_norm) > 2:
                            pending_norm.pop(0)()

                # round end: flush fillers that later rounds depend on
                while p1q:
                    p1q.popleft()()
                while p3q:
                    p3q.popleft()()
                while pending_norm:
                    pending_norm.pop(0)()

            # ---- tail: output projection of block 3 ----
            for q in p3_quanta(NBLK - 1, "act"):
                q()

    nc.compile()
    return nc


def _host_inputs(x, q_proj, k_proj, v_proj, o_proj):
    """Per-core input dicts (numpy, bf16, packed to SBUF tile layouts)."""
    import ml_dtypes

    bf16 = ml_dtypes.bfloat16
    masks = np.zeros((128, 2, 128), dtype=np.float32)
    jj = np.arange(128)[:, None]
    ii = np.arange(128)[None, :]
    tri = np.where(jj <= ii, 0.0, NEG)
    masks[:, 0, :] = tri
    masks[:, 1, :] = tri
    masks = masks.astype(bf16)

    # x[b].T is [D, T]; pack to [128p, NBLK, KT, BLK]
    xw = []
    for b in range(B):
        xT = np.ascontiguousarray(x[b].T).astype(bf16)
        xw.append(
            np.ascontiguousarray(
                xT.reshape(KT, 128, NBLK, BLK).transpose(1, 2, 0, 3)
            )
        )
    in_maps = []
    for c in range(N_CORES):
        b, g = divmod(c, GROUPS)
        wqkv = np.concatenate(
            [
                q_proj[QCOLS * g : QCOLS * g + QCOLS].T,
                k_proj[KCOLS * g : KCOLS * g + KCOLS].T,
                v_proj[KCOLS * g : KCOLS * g + KCOLS].T,
            ],
            axis=1,
        ).astype(bf16)  # [D, WCOLS]
        # pack to [128p, 6m, KT, 128]
        wq = np.ascontiguousarray(
            wqkv.reshape(KT, 128, 6, 128).transpose(1, 2, 0, 3)
        )
        opj = np.ascontiguousarray(
            o_proj[:, QCOLS * g : QCOLS * g + QCOLS].T
        ).astype(bf16)  # [QCOLS, D]
        op = np.ascontiguousarray(opj.reshape(4, 128, D).transpose(1, 0, 2))
        in_maps.append(
            {"xw": xw[b], "wq": wq, "op": op, "masks": masks}
        )
    return in_maps


def run(x, q_proj, k_proj, v_proj, o_proj, trace=False):
    """Run on hardware; returns (output [B,T,D] f32, BassKernelResults)."""
    if "nc" not in _cache:
        _cache["nc"] = _build()
    nc = _cache["nc"]
    in_maps = _host_inputs(x, q_proj, k_proj, v_proj, o_proj)
    res = run_bass_kernel_spmd(
        nc, in_maps, core_ids=list(range(N_CORES)), trace=trace
    )
    out = np.empty((B, T, D), dtype=np.float32)
    for b in range(B):
        acc = res.results[4 * b]["out"].astype(np.float32)
        for g in range(1, GROUPS):
            acc += res.results[4 * b + g]["out"].astype(np.float32)
        out[b] = acc.reshape(T, D)
    return out, res


def kernel(x, q_proj, k_proj, v_proj, o_proj, hq=None, hk=None, **_unused):
    x = np.asarray(x, dtype=np.float32)
    q_proj = np.asarray(q_proj, dtype=np.float32)
    k_proj = np.asarray(k_proj, dtype=np.float32)
    v_proj = np.asarray(v_proj, dtype=np.float32)
    o_proj = np.asarray(o_proj, dtype=np.float32)
    assert x.shape == (B, T, D), x.shape
    trace = bool(os.environ.get("KERNEL_TRACE"))
    out, _ = run(x, q_proj, k_proj, v_proj, o_proj, trace=trace)
    return out
